# revision 2
# baseline (speedup 1.0000x reference)
"""Trainium2 Bass kernel for nn_Critic (gnn_message_passing).

Strategy (pure data-parallel over batch, 8 cores x 128 rows):

The reference attention is algebraically collapsed: for single-query
attention, q.(feat@Wk) == feat.(Wk@q), so instead of materializing
[B,N,V] key/value projections we compute a per-row 14-vector
qk[b] = ego'[b] @ (Wq @ Wk^T) and score s[b,n] = feat[b,n,:] . qk[b].
Similarly out = (softmax @ feat) @ Wv, pooling feat first (14 dims).

The subject-bus id subtraction (cols 0/7) shifts scores by a per-row
constant -> cancels in softmax; its effect on the pooled output is a
rank-1 term  -subj_id * sum(w) * (Wv[0]+Wv[7]), appended as an extra
contraction row.  BatchNorm (training mode, global batch stats) needs a
cross-core AllReduce of per-feature sum/sumsq; the BN affine is then
folded into the head-MLP weights (W1' = diag(s)@W1, b1' = b1 + t0@W1),
so no elementwise BN over activations is ever done.

elu(x) is composed as relu(x) + min(exp(x),1) - 1 with the -1 folded
into the scalar output bias via column sums of t_W2.
"""

import numpy as np
from contextlib import ExitStack

import concourse.bacc as bacc
import concourse.tile as tile
from concourse import mybir
import concourse.bass as bass
from concourse.bass_utils import run_bass_kernel_spmd
from concourse.masks import make_identity

B, N, V = 1024, 256, 200
NC = 8
BS = B // NC  # 128 rows per core
F32 = mybir.dt.float32
ALU = mybir.AluOpType
ACTF = mybir.ActivationFunctionType
SCALE = float(1.0 / np.sqrt(V))
NEG = -1.0e9

_cache = {}


def build_nc():
    import os
    STAGE = int(os.environ.get("K_STAGE", "9"))
    nc = bacc.Bacc(None)

    # ---- kernel I/O ----
    mp = nc.dram_tensor("mp", [BS, 15 * N], F32, kind="ExternalInput")  # planes
    egoT = nc.dram_tensor("egoT", [10, BS], F32, kind="ExternalInput")
    wqT = nc.dram_tensor("wqT", [V, 21], F32, kind="ExternalInput")
    wkT = nc.dram_tensor("wkT", [V, 35], F32, kind="ExternalInput")
    wv = nc.dram_tensor("wv", [14, 600], F32, kind="ExternalInput")
    wv07 = nc.dram_tensor("wv07", [1, 1200], F32, kind="ExternalInput")
    w1 = nc.dram_tensor("w1", [V, 600], F32, kind="ExternalInput")
    w2 = nc.dram_tensor("w2", [V, 3], F32, kind="ExternalInput")
    b1T = nc.dram_tensor("b1T", [V, 3], F32, kind="ExternalInput")
    ew1 = nc.dram_tensor("ew1", [4, V], F32, kind="ExternalInput")
    ew2 = nc.dram_tensor("ew2", [V, V], F32, kind="ExternalInput")
    ew3 = nc.dram_tensor("ew3", [V, 1], F32, kind="ExternalInput")
    eb1T = nc.dram_tensor("eb1T", [V, 1], F32, kind="ExternalInput")
    eb2T = nc.dram_tensor("eb2T", [V, 1], F32, kind="ExternalInput")
    gamT = nc.dram_tensor("gamT", [V, 1], F32, kind="ExternalInput")
    betT = nc.dram_tensor("betT", [V, 1], F32, kind="ExternalInput")
    bsum4 = nc.dram_tensor("bsum4", [1, 4], F32, kind="ExternalInput")
    out = nc.dram_tensor("out", [BS, 1], F32, kind="ExternalOutput")

    VC = [(0, 128), (128, 200)]  # v-dim chunks

    with tile.TileContext(nc) as tc:
        with ExitStack() as ctx:
            sb1 = ctx.enter_context(tc.tile_pool(name="sb1", bufs=1))
            ps = ctx.enter_context(tc.tile_pool(name="ps", bufs=2, space="PSUM"))
            ps3 = ctx.enter_context(tc.tile_pool(name="ps3", bufs=3, space="PSUM"))
            psg = ctx.enter_context(tc.tile_pool(name="psg", bufs=1, space="PSUM"))
            dram = ctx.enter_context(tc.tile_pool(name="dram", bufs=1, space="DRAM"))

            # ---------------- DMA in ----------------
            # per-plane tiles for fine-grained overlap
            planes = []
            for f in [2, 14, 0, 1, 3, 4, 5, 6, 7, 8, 9, 10, 11, 12, 13]:
                pl = sb1.tile([BS, N], F32, tag=f"pl{f}", name=f"pl{f}")
                nc.sync.dma_start(out=pl, in_=mp[:, f * N:(f + 1) * N])
                planes.append((f, pl))
            planes = dict(planes)
            loc, flag = planes[2], planes[14]

            ego_sb = sb1.tile([6, BS], F32)
            nc.sync.dma_start(out=ego_sb, in_=egoT[0:6, :])
            egoM_sb = sb1.tile([4, BS], F32)
            nc.sync.dma_start(out=egoM_sb, in_=egoT[6:10, :])
            wqT_sb = [sb1.tile([c1 - c0, 21], F32, tag=f"wq{i}", name=f"wq{i}") for i, (c0, c1) in enumerate(VC)]
            wkT_sb = [sb1.tile([c1 - c0, 35], F32, tag=f"wk{i}", name=f"wk{i}") for i, (c0, c1) in enumerate(VC)]
            for i, (c0, c1) in enumerate(VC):
                nc.sync.dma_start(out=wqT_sb[i], in_=wqT[c0:c1, :])
                nc.sync.dma_start(out=wkT_sb[i], in_=wkT[c0:c1, :])
            wv_sb = sb1.tile([15, 600], F32)
            nc.sync.dma_start(out=wv_sb[0:14, :], in_=wv[:])
            wv07_sb = sb1.tile([1, 1200], F32)
            nc.sync.dma_start(out=wv07_sb, in_=wv07[:])
            w1_sb = [sb1.tile([c1 - c0, 600], F32, tag=f"w1{i}", name=f"w1{i}") for i, (c0, c1) in enumerate(VC)]
            w2_sb = [sb1.tile([c1 - c0, 3], F32, tag=f"w2{i}", name=f"w2{i}") for i, (c0, c1) in enumerate(VC)]
            b1T_sb = [sb1.tile([c1 - c0, 3], F32, tag=f"b1T{i}", name=f"b1T{i}") for i, (c0, c1) in enumerate(VC)]
            ew2_sb = [sb1.tile([c1 - c0, V], F32, tag=f"ew2{i}", name=f"ew2{i}") for i, (c0, c1) in enumerate(VC)]
            ew3_sb = [sb1.tile([c1 - c0, 1], F32, tag=f"ew3{i}", name=f"ew3{i}") for i, (c0, c1) in enumerate(VC)]
            eb1T_sb = [sb1.tile([c1 - c0, 1], F32, tag=f"eb1{i}", name=f"eb1{i}") for i, (c0, c1) in enumerate(VC)]
            eb2T_sb = [sb1.tile([c1 - c0, 1], F32, tag=f"eb2{i}", name=f"eb2{i}") for i, (c0, c1) in enumerate(VC)]
            gamT_sb = [sb1.tile([c1 - c0, 1], F32, tag=f"gam{i}", name=f"gam{i}") for i, (c0, c1) in enumerate(VC)]
            betT_sb = [sb1.tile([c1 - c0, 1], F32, tag=f"bet{i}", name=f"bet{i}") for i, (c0, c1) in enumerate(VC)]
            for i, (c0, c1) in enumerate(VC):
                nc.sync.dma_start(out=w1_sb[i], in_=w1[c0:c1, :])
                nc.sync.dma_start(out=w2_sb[i], in_=w2[c0:c1, :])
                nc.sync.dma_start(out=b1T_sb[i], in_=b1T[c0:c1, :])
                nc.sync.dma_start(out=ew2_sb[i], in_=ew2[c0:c1, :])
                nc.sync.dma_start(out=ew3_sb[i], in_=ew3[c0:c1, :])
                nc.sync.dma_start(out=eb1T_sb[i], in_=eb1T[c0:c1, :])
                nc.sync.dma_start(out=eb2T_sb[i], in_=eb2T[c0:c1, :])
                nc.sync.dma_start(out=gamT_sb[i], in_=gamT[c0:c1, :])
                nc.sync.dma_start(out=betT_sb[i], in_=betT[c0:c1, :])
            ew1_sb = sb1.tile([4, V], F32)
            nc.sync.dma_start(out=ew1_sb, in_=ew1[:])
            bsum4_sb = sb1.tile([1, 4], F32)
            nc.sync.dma_start(out=bsum4_sb, in_=bsum4[:])

            ident = sb1.tile([128, 128], F32)
            make_identity(nc, ident)
            ones_col = sb1.tile([128, 1], F32)
            nc.gpsimd.memset(ones_col, 1.0)
            ones_row = sb1.tile([1, 128], F32)
            nc.gpsimd.memset(ones_row, 1.0)
            eps_col = sb1.tile([128, 1], F32)
            nc.gpsimd.memset(eps_col, 1.0e-5)

            # ---------------- query chain (PE) ----------------
            # Wcomb'[6,35]: rows = ego cols 1..6 of (Wq @ Wk^T) * SCALE
            wc_ps = ps.tile([6, 35], F32, tag="sm", name="wc_ps")
            segcols = [(0, 0, 14), (7, 14, 28), (14, 28, 35)]  # (wq col, wk c0, wk c1)
            for si, (qc, k0, k1) in enumerate(segcols):
                for i in range(2):
                    nc.tensor.matmul(
                        wc_ps[:, k0:k1], wqT_sb[i][:, qc + 1:qc + 7], wkT_sb[i][:, k0:k1],
                        start=(i == 0), stop=(i == 1))
            wc_sb = sb1.tile([6, 35], F32)
            nc.scalar.activation(wc_sb, wc_ps, ACTF.Copy, bias=0.0, scale=SCALE)

            # qk[128,35] = ego'[cols 1..5, a] @ Wcomb'
            qk_ps = ps.tile([BS, 35], F32, tag="sm", name="qk_ps")
            nc.tensor.matmul(qk_ps, ego_sb, wc_sb, start=True, stop=True)
            qk_sb = sb1.tile([BS, 35], F32)
            nc.scalar.activation(qk_sb, qk_ps, ACTF.Copy, bias=0.0, scale=1.0)

            # ---------------- masks (DVE) -> score accumulators ----------------
            subj_loc = loc[:, 0:1]
            geM = sb1.tile([BS, N], F32)
            nc.vector.tensor_scalar(geM, loc, subj_loc, NEG, op0=ALU.is_ge, op1=ALU.mult)
            leM = sb1.tile([BS, N], F32)
            nc.vector.tensor_scalar(leM, loc, subj_loc, NEG, op0=ALU.is_le, op1=ALU.mult)
            nfM = sb1.tile([BS, N], F32)
            nc.vector.tensor_scalar(nfM, flag, 1.0e9, NEG, op0=ALU.mult, op1=ALU.add)
            acc = {}
            acc['u'] = sb1.tile([BS, N], F32, tag="accu", name="accu")
            nc.vector.tensor_tensor(acc['u'], geM, nfM, op=ALU.min)
            acc['d'] = sb1.tile([BS, N], F32, tag="accd", name="accd")
            nc.vector.tensor_tensor(acc['d'], leM, nfM, op=ALU.min)
            acc['p'] = sb1.tile([BS, N], F32, tag="accp", name="accp")
            nc.vector.tensor_scalar(acc['p'], flag, NEG, None, op0=ALU.mult)

            # ---------------- scores (DVE STT) ----------------
            SEG = [('u', 14, 0), ('d', 14, 14), ('p', 7, 28)]
            for s, nf, j0 in SEG:
                for f in range(nf):
                    nc.vector.scalar_tensor_tensor(
                        acc[s], planes[f], qk_sb[:, j0 + f:j0 + f + 1], acc[s],
                        op0=ALU.mult, op1=ALU.add)

            if STAGE <= 1:
                g_sb = sb1.tile([BS, 1], F32, name="g_sb")
                nc.vector.tensor_copy(g_sb, acc['u'][:, 0:1])
                nc.sync.dma_start(out=out[:], in_=g_sb)
                return nc
            # ---------------- softmax exp (ACT) + recip (DVE) ----------------
            w_t, rs_t, wsum1 = {}, {}, {}
            for s, nf, j0 in SEG:
                w_t[s] = sb1.tile([BS, N], F32, tag=f"w{s}", name=f"w{s}")
                se = sb1.tile([BS, 1], F32, tag=f"se{s}", name=f"se{s}")
                nc.scalar.activation(w_t[s], acc[s], ACTF.Exp, bias=0.0, scale=1.0,
                                     accum_out=se)
                seb = sb1.tile([BS, 1], F32, tag=f"seb{s}", name=f"seb{s}")
                nc.vector.tensor_scalar_add(seb, se, 1.0e-30)
                rs_t[s] = sb1.tile([BS, 1], F32, tag=f"rs{s}", name=f"rs{s}")
                nc.vector.reciprocal(rs_t[s], seb)
                wsum1[s] = sb1.tile([BS, 1], F32, tag=f"ws{s}", name=f"ws{s}")
                nc.vector.tensor_tensor(wsum1[s], se, rs_t[s], op=ALU.mult)

            if STAGE <= 2:
                g_sb = sb1.tile([BS, 1], F32, name="g_sb")
                nc.vector.tensor_copy(g_sb, rs_t['u'])
                nc.sync.dma_start(out=out[:], in_=g_sb)
                return nc
            # ---------------- pooled (DVE TTR) ----------------
            scr = sb1.tile([BS, N], F32)  # throwaway elementwise product
            pool = {}
            for s, nf, j0 in SEG:
                pool[s] = sb1.tile([BS, 16], F32, tag=f"pool{s}", name=f"pool{s}")
                for f in range(nf):
                    nc.vector.scalar_tensor_tensor(
                        scr, planes[f], 1.0, w_t[s],
                        op0=ALU.mult, op1=ALU.mult,
                        accum_out=pool[s][:, f:f + 1])
                # normalize + subject row (= subj_id * sum(w_norm))
                nc.vector.tensor_scalar_mul(pool[s][:, 0:nf], pool[s][:, 0:nf], rs_t[s])
                if nf < 14:
                    nc.vector.memset(pool[s][:, nf:14], 0.0)
                nc.vector.tensor_tensor(pool[s][:, 14:15], planes[0][:, 0:1], wsum1[s],
                                        op=ALU.mult)
                nc.vector.memset(pool[s][:, 15:16], 0.0)

            # Wv extension row: -(Wv[0] + Wv[7]) (pv cols of row7 are zero)
            ext_t = sb1.tile([1, 600], F32)
            nc.vector.scalar_tensor_tensor(
                ext_t, wv07_sb[:, 0:600], -1.0, wv07_sb[:, 600:1200],
                op0=ALU.mult, op1=ALU.subtract)
            nc.sync.dma_start(out=wv_sb[14:15, :], in_=ext_t)

            if STAGE <= 3:
                g_sb = sb1.tile([BS, 1], F32, name="g_sb")
                nc.vector.tensor_copy(g_sb, pool['u'][:, 0:1])
                nc.sync.dma_start(out=out[:], in_=g_sb)
                return nc
            # ---------------- pooled^T, u/d/p, stats ----------------
            UU = sb1.tile([BS, 1216], F32)
            nc.vector.memset(UU[:, 1200:1216], 0.0)
            poolT_sb = {}
            xT = {}
            for si, (s, nf, j0) in enumerate(SEG):
                pT = ps.tile([16, BS], F32, tag="sm", name="pT")
                nc.tensor.transpose(pT, pool[s], ident)
                poolT_sb[s] = sb1.tile([16, BS], F32, tag=f"pT{s}", name=f"pT{s}")
                nc.scalar.activation(poolT_sb[s], pT, ACTF.Copy, bias=0.0, scale=1.0)
                # u in [b, v] for stats
                ups = ps3.tile([BS, V], F32, tag="big", name="ups")
                nc.tensor.matmul(ups, poolT_sb[s][0:15, :], wv_sb[:, si * V:(si + 1) * V],
                                 start=True, stop=True)
                nc.scalar.activation(UU[:, si * V:(si + 1) * V], ups, ACTF.Copy,
                                     bias=0.0, scale=1.0)
                # u^T in [v, b] for the head MLP (BN folded into weights later)
                xT[s] = []
                for i, (c0, c1) in enumerate(VC):
                    xps = ps3.tile([c1 - c0, BS], F32, tag="big", name="xps")
                    nc.tensor.matmul(xps, wv_sb[:, si * V + c0:si * V + c1],
                                     poolT_sb[s][0:15, :], start=True, stop=True)
                    xsb = sb1.tile([c1 - c0, BS], F32, tag=f"xT{s}{i}", name=f"xT{s}{i}")
                    nc.scalar.activation(xsb, xps, ACTF.Copy, bias=0.0, scale=1.0)
                    xT[s].append(xsb)

            nc.vector.tensor_tensor(UU[:, 600:1200], UU[:, 0:600], UU[:, 0:600],
                                    op=ALU.mult)

            if STAGE <= 4:
                g_sb = sb1.tile([BS, 1], F32, name="g_sb")
                nc.vector.tensor_copy(g_sb, UU[:, 0:1])
                nc.sync.dma_start(out=out[:], in_=g_sb)
                return nc
            # column sums, transposed: st2[v-part, j] via matmul(UU-cols, ones)
            in_b = dram.tile([V, 6], F32)
            st_sb = []
            for i, (c0, c1) in enumerate(VC):
                pc = c1 - c0
                stp = ps.tile([pc, 6], F32, tag="sm", name=f"stp{i}")
                for j in range(6):
                    nc.tensor.matmul(stp[:, j:j + 1], UU[:, j * 200 + c0:j * 200 + c1],
                                     ones_col, start=True, stop=True)
                t = sb1.tile([pc, 6], F32, tag=f"stsb{i}", name=f"stsb{i}")
                nc.vector.tensor_copy(t, stp)
                st_sb.append(t)
                nc.sync.dma_start(out=in_b[c0:c1, :], in_=t)
            out_b = dram.tile([V, 6], F32, addr_space="Shared")
            if __import__("os").environ.get("NO_CC"):
                nc.sync.dma_start(out=out_b[:], in_=in_b[:])
            else:
                nc.gpsimd.collective_compute(
                    "AllReduce", ALU.add, ins=[in_b[:]], outs=[out_b[:]],
                    replica_groups=[list(range(NC))])

            if STAGE <= 5:
                g_sb = sb1.tile([BS, 1], F32, name="g_sb")
                nc.vector.tensor_copy(g_sb, st_sb[0][:, 0:1])
                nc.sync.dma_start(out=out[:], in_=g_sb)
                return nc
            # ---------------- BN affine from global stats ----------------
            # stat2[v-part, j]: j in {sum_u, sum_d, sum_p, sq_u, sq_d, sq_p}
            s3_t, t03_t = [], []
            for i, (c0, c1) in enumerate(VC):
                pc = c1 - c0
                st = sb1.tile([pc, 6], F32, tag=f"st{i}", name=f"st{i}")
                nc.sync.dma_start(out=st, in_=out_b[c0:c1, :])
                nc.vector.tensor_scalar_mul(st, st, 1.0 / B)  # means
                sq = sb1.tile([pc, 3], F32, tag=f"sq{i}", name=f"sq{i}")
                nc.vector.tensor_tensor(sq, st[:, 0:3], st[:, 0:3], op=ALU.mult)
                var = sb1.tile([pc, 3], F32, tag=f"var{i}", name=f"var{i}")
                nc.vector.tensor_tensor(var, st[:, 3:6], sq, op=ALU.subtract)
                std = sb1.tile([pc, 3], F32, tag=f"std{i}", name=f"std{i}")
                nc.scalar.activation(std, var, ACTF.Sqrt, bias=eps_col[0:pc, :], scale=1.0)
                rstd = sb1.tile([pc, 3], F32, tag=f"rstd{i}", name=f"rstd{i}")
                nc.vector.reciprocal(rstd, std)
                gam_b = bass.AP(tensor=gamT_sb[i].tensor, offset=gamT_sb[i].offset,
                                ap=[gamT_sb[i].ap[0], [0, 3]])
                bet_b = bass.AP(tensor=betT_sb[i].tensor, offset=betT_sb[i].offset,
                                ap=[betT_sb[i].ap[0], [0, 3]])
                s3 = sb1.tile([pc, 3], F32, tag=f"s3{i}", name=f"s3{i}")
                nc.vector.tensor_tensor(s3, rstd, gam_b, op=ALU.mult)
                z3 = sb1.tile([pc, 3], F32, tag=f"z3{i}", name=f"z3{i}")
                nc.vector.tensor_tensor(z3, st[:, 0:3], s3, op=ALU.mult)
                t03 = sb1.tile([pc, 3], F32, tag=f"t03{i}", name=f"t03{i}")
                nc.vector.tensor_tensor(t03, bet_b, z3, op=ALU.subtract)
                s3_t.append(s3)
                t03_t.append(t03)

            # W1' = diag(s) @ W1  (per head, per v-chunk)
            w1p = []
            for i, (c0, c1) in enumerate(VC):
                t = sb1.tile([c1 - c0, 600], F32, tag=f"w1p{i}", name=f"w1p{i}")
                for k in range(3):
                    nc.vector.tensor_scalar_mul(
                        t[:, k * V:(k + 1) * V], w1_sb[i][:, k * V:(k + 1) * V],
                        s3_t[i][:, k:k + 1])
                w1p.append(t)

            # b1' = b1 + t0 @ W1 (raw W1), computed transposed [w,1] per head
            B1 = []
            for j, (w0, w1c) in enumerate(VC):
                pc = w1c - w0
                bt = sb1.tile([pc, 3], F32, tag=f"B1{j}", name=f"B1{j}")
                for k in range(3):
                    bp = ps.tile([pc, 1], F32, tag="sm", name="bp")
                    for i in range(2):
                        nc.tensor.matmul(bp, w1_sb[i][:, k * V + w0:k * V + w1c],
                                         t03_t[i][:, k:k + 1],
                                         start=(i == 0), stop=(i == 1))
                    nc.vector.tensor_copy(bt[:, k:k + 1], bp)
                nc.vector.tensor_tensor(bt, bt, b1T_sb[j], op=ALU.add)
                B1.append(bt)

            if STAGE <= 6:
                g_sb = sb1.tile([BS, 1], F32, name="g_sb")
                nc.vector.tensor_copy(g_sb, w1p[0][:, 0:1])
                nc.sync.dma_start(out=out[:], in_=g_sb)
                return nc
            # ---------------- G accumulation (heads + ego MLP + biases) ----------------
            # ego-MLP (independent of collective): q1 = relu(ego_t@eW1+eb1)
            q1T, q2T = [], []
            for j, (w0, w1c) in enumerate(VC):
                pc = w1c - w0
                qp = ps3.tile([pc, BS], F32, tag="big", name="qp")
                nc.tensor.matmul(qp, ew1_sb[:, w0:w1c], egoM_sb,
                                 start=True, stop=True)
                qs = sb1.tile([pc, BS], F32, tag=f"q1T{j}", name=f"q1T{j}")
                nc.scalar.activation(qs, qp, ACTF.Relu, bias=eb1T_sb[j], scale=1.0)
                q1T.append(qs)
            for j, (w0, w1c) in enumerate(VC):
                pc = w1c - w0
                qp = ps3.tile([pc, BS], F32, tag="big", name="qp2")
                for i in range(2):
                    nc.tensor.matmul(qp, ew2_sb[i][:, w0:w1c], q1T[i],
                                     start=(i == 0), stop=(i == 1))
                qs = sb1.tile([pc, BS], F32, tag=f"q2T{j}", name=f"q2T{j}")
                nc.scalar.activation(qs, qp, ACTF.Relu, bias=eb2T_sb[j], scale=1.0)
                q2T.append(qs)

            G = psg.tile([BS, 1], F32)
            nmm = 0
            # Q1 = q2 @ eW3  (2 matmuls)
            for i in range(2):
                nc.tensor.matmul(G, q2T[i], ew3_sb[i], start=(nmm == 0), stop=False,
                                 skip_group_check=True)
                nmm += 1

            # bias constant: sum(b2)+eb3 - sum_k sum_w W2[w,k]
            wsp = ps.tile([1, 3], F32, tag="sm", name="wsp")
            for i in range(2):
                nc.tensor.matmul(wsp, ones_col[0:VC[i][1] - VC[i][0], :], w2_sb[i],
                                 start=(i == 0), stop=(i == 1))
            wss = sb1.tile([1, 3], F32)
            nc.vector.tensor_copy(wss, wsp)
            r1 = sb1.tile([1, 1], F32)
            nc.vector.reduce_sum(r1, bsum4_sb, axis=mybir.AxisListType.X)
            r2 = sb1.tile([1, 1], F32)
            nc.vector.reduce_sum(r2, wss, axis=mybir.AxisListType.X)
            bs_tot = sb1.tile([1, 1], F32)
            nc.vector.tensor_tensor(bs_tot, r1, r2, op=ALU.subtract)
            nc.tensor.matmul(G, ones_row, bs_tot, start=False, stop=False,
                             skip_group_check=True)
            nmm += 1

            # heads: hT = elu(W1'^T @ xT + b1') + 1 (the +1 folded into bias const)
            for k, s in enumerate(['u', 'd', 'p']):
                for j, (w0, w1c) in enumerate(VC):
                    pc = w1c - w0
                    hp = ps3.tile([pc, BS], F32, tag="big", name="hp")
                    for i in range(2):
                        nc.tensor.matmul(hp, w1p[i][:, k * V + w0:k * V + w1c],
                                         xT[s][i], start=(i == 0), stop=(i == 1))
                    eh = sb1.tile([pc, BS], F32, tag=f"eh{j}", name=f"eh{j}")
                    nc.scalar.activation(eh, hp, ACTF.Exp, bias=B1[j][:, k:k + 1],
                                         scale=1.0)
                    rh = sb1.tile([pc, BS], F32, tag=f"rh{j}", name=f"rh{j}")
                    nc.scalar.activation(rh, hp, ACTF.Relu, bias=B1[j][:, k:k + 1],
                                         scale=1.0)
                    ht = sb1.tile([pc, BS], F32, tag=f"ht{j}", name=f"ht{j}")
                    nc.vector.scalar_tensor_tensor(ht, eh, 1.0, rh,
                                                   op0=ALU.min, op1=ALU.add)
                    nc.tensor.matmul(G, ht, w2_sb[j][:, k:k + 1], start=False,
                                     stop=(k == 2 and j == 1), skip_group_check=True)
                    nmm += 1

            g_sb = sb1.tile([BS, 1], F32)
            nc.vector.tensor_copy(g_sb, G)
            nc.sync.dma_start(out=out[:], in_=g_sb)

    nc.finalize()
    return nc


def prep_inputs(inputs):
    """Host-side layout-only prep (shard, transpose, concat, pad)."""
    merged = np.ascontiguousarray(inputs["merged"], dtype=np.float32)
    a = np.ascontiguousarray(inputs["a"], dtype=np.float32)

    up_Wq, up_Wk, up_Wv = inputs["up_Wq"], inputs["up_Wk"], inputs["up_Wv"]
    dn_Wq, dn_Wk, dn_Wv = inputs["dn_Wq"], inputs["dn_Wk"], inputs["dn_Wv"]
    pv_Wq, pv_Wk, pv_Wv = inputs["pv_Wq"], inputs["pv_Wk"], inputs["pv_Wv"]
    t_W1, t_b1, t_W2, t_b2 = inputs["t_W1"], inputs["t_b1"], inputs["t_W2"], inputs["t_b2"]
    e_W1, e_b1, e_W2, e_b2 = inputs["e_W1"], inputs["e_b1"], inputs["e_W2"], inputs["e_b2"]
    e_W3, e_b3 = inputs["e_W3"], inputs["e_b3"]
    gamma, beta = inputs["gamma"], inputs["beta"]

    f32 = lambda x: np.ascontiguousarray(x, dtype=np.float32)
    wqT = f32(np.concatenate([up_Wq.T, dn_Wq.T, pv_Wq.T], axis=1))        # [200,21]
    wkT = f32(np.concatenate([up_Wk.T, dn_Wk.T, pv_Wk.T], axis=1))        # [200,35]
    pvv = np.zeros((14, V), np.float32)
    pvv[0:7] = pv_Wv
    wv = f32(np.concatenate([up_Wv, dn_Wv, pvv], axis=1))                 # [14,600]
    wv07 = f32(np.concatenate([wv[0], wv[7]]))[None, :]                   # [1,1200]
    w1 = f32(np.concatenate([t_W1[0], t_W1[1], t_W1[2]], axis=1))         # [200,600]
    w2 = f32(t_W2[:, :, 0].T)                                             # [200,3]
    b1T = f32(t_b1.T)                                                     # [200,3]
    ew1 = f32(e_W1)
    ew2 = f32(e_W2)
    ew3 = f32(e_W3)
    eb1T = f32(e_b1[:, None])
    eb2T = f32(e_b2[:, None])
    gamT = f32(gamma[:, None])
    betT = f32(beta[:, None])
    bsum4 = f32(np.concatenate([t_b2[:, 0], e_b3]))[None, :]              # [1,4]

    shared = dict(wqT=wqT, wkT=wkT, wv=wv, wv07=wv07, w1=w1, w2=w2, b1T=b1T, ew1=ew1,
                  ew2=ew2, ew3=ew3, eb1T=eb1T, eb2T=eb2T, gamT=gamT, betT=betT,
                  bsum4=bsum4)

    in_maps = []
    for c in range(NC):
        sh = merged[c * BS:(c + 1) * BS]                                  # [128,256,15]
        mp = f32(sh.transpose(0, 2, 1).reshape(BS, 15 * N))               # planes
        egoT = np.zeros((10, BS), np.float32)
        egoT[0:5] = sh[:, 0, 1:6].T
        egoT[5] = a[c * BS:(c + 1) * BS]
        egoT[6:9] = sh[:, 0, 3:6].T
        egoT[9] = a[c * BS:(c + 1) * BS]
        m = dict(shared)
        m["mp"] = mp
        m["egoT"] = f32(egoT)
        in_maps.append(m)
    return in_maps


def _build():
    nc = build_nc()
    if not nc.is_finalized():
        nc.finalize()
    return nc


def kernel(**inputs):
    if "nc" not in _cache:
        _cache["nc"] = _build()
    nc = _cache["nc"]
    in_maps = prep_inputs(inputs)
    r = run_bass_kernel_spmd(nc, in_maps, list(range(NC)), trace=False)
    _cache["last"] = r
    out = np.concatenate([r.results[c]["out"] for c in range(NC)], axis=0)
    return out.reshape(-1, 1).astype(np.float32)



# revision 18
# speedup vs baseline: 1.1565x; 1.1565x over previous
"""Trainium2 Bass kernel for nn_Critic (gnn_message_passing).

Strategy (pure data-parallel over batch, 8 cores x 128 rows):

Single-query attention is algebraically collapsed: score s[b,n] =
feat[b,n,:] . qk[b,:] with qk = ego' @ (Wq @ Wk^T * scale) (the Wq@Wk^T
product is a weight-only constant, folded on host).  Pooling is done in
feature space (14 dims) before the Wv projection.  The subject-bus id
subtraction shifts scores by a per-row constant (cancels in softmax);
its pooled effect is a rank-1 term handled by an extra contraction row
whose weights -(Wv[0]+Wv[7]) are folded on host.

BatchNorm batch stats are computed WITHOUT materializing x=[B,200]:
with pool~[b,0:15] the pooled features (+subj term) and pool~[b,15]=1,
the Gram matrix Gt = pool~^T pool~ [16,16] per head carries all first
and second moments: sum_b x[:,v] = P . wv[:,v] (P = Gt[:,15]) and
sum_b x[:,v]^2 = wv[:,v]^T G wv[:,v].  Only the three 16x16 Gram
matrices are AllReduced ([16,48] = 3KB), right after pooling.
Post-collective, per-feature sums come from tiny K=15 matmuls, and
1/sqrt(var+eps) is exp(-0.5*ln(var+eps)) so the Scalar engine needs a
single activation-table set (natural_log_exp_and_others) all kernel.

The BN affine is folded into the head-MLP inputs (x' = s3*x, bias via
b1' = b1 + t0@W1), elu(x) = relu(x) + min(exp(x),1) - 1 with the -1
folded into a single host-side output-bias constant.

Engine split: the per-feature score/pool accumulation loops (the bulk
elementwise work, [128,256] each) run on BOTH the Vector and GpSimd
engines (u/p chains on DVE, d chain on Pool), in bf16 for full DVE
rate and half DMA bytes.  Input DMAs are coalesced into 5 transfers on
two HWDGE queues (sync + scalar).
"""

import os
import numpy as np
from contextlib import ExitStack

import ml_dtypes
import concourse.bacc as bacc
import concourse.tile as tile
from concourse import mybir
import concourse.bass as bass
from concourse.bass_utils import run_bass_kernel_spmd
from concourse.masks import make_identity

B, N, V = 1024, 256, 200
NC = 8
BS = B // NC  # 128 rows per core
F32 = mybir.dt.float32
BF16 = mybir.dt.bfloat16
ALU = mybir.AluOpType
ACTF = mybir.ActivationFunctionType
SCALE = float(1.0 / np.sqrt(V))
NEG = -1.0e9

VC = [(0, 128), (128, 200)]  # v-dim chunks

# tall weight tensor column layout ([128, C_TALL] f32, 200-row weights
# packed as chunk0 rows 0:128 | chunk1 rows 0:72)
W1 = 0        # 2 x 600 (t_W1 heads concatenated: k*200+v)
EW2 = 1200    # 2 x 200
W2 = 1600     # 2 x 3
B1T = 1606    # 2 x 3
EW3 = 1612    # 2 x 1
EB1 = 1614    # 2 x 1
EB2 = 1616    # 2 x 1
GAM = 1618    # 2 x 1
BET = 1620    # 2 x 1
C_TALL = 1622

# short weight tensor ([16, C_SHORT] f32)
WV = 0        # 600 (rows 0:14 Wv concat, row14 = -(Wv[0]+Wv[7]), row15 = 0)
WC = 600      # 35 (rows 0:6: (Wq @ Wk^T)[1:7] * scale, segs u|d|p)
EW1 = 635     # 200 (rows 0:4)
CB = 835      # 1 (row 0: sum(t_b2) + e_b3 - sum(t_W2))
C_SHORT = 836

_cache = {}


def build_nc():
    STAGE = int(os.environ.get("K_STAGE", "9"))
    AG = bool(os.environ.get("K_AG"))  # AllGather + local reduce instead of AllReduce
    nc = bacc.Bacc(None)

    lf = nc.dram_tensor("lf", [BS, 2 * N + 1], F32, kind="ExternalInput")
    mp = nc.dram_tensor("mp", [BS, 14 * N], BF16, kind="ExternalInput")
    tallt = nc.dram_tensor("tall", [128, C_TALL], F32, kind="ExternalInput")
    shortt = nc.dram_tensor("short", [16, C_SHORT], F32, kind="ExternalInput")
    egoT = nc.dram_tensor("egoT", [10, BS], F32, kind="ExternalInput")
    out = nc.dram_tensor("out", [BS, 1], F32, kind="ExternalOutput")

    SEG = [('u', 14, 0), ('d', 14, 14), ('p', 7, 28)]

    with tile.TileContext(nc) as tc:
        with ExitStack() as ctx:
            sb = ctx.enter_context(tc.tile_pool(name="sb", bufs=1))
            ps_sm = ctx.enter_context(tc.tile_pool(name="ps_sm", bufs=2, space="PSUM"))
            ps_big = ctx.enter_context(tc.tile_pool(name="ps_big", bufs=3, space="PSUM"))
            psg = ctx.enter_context(tc.tile_pool(name="psg", bufs=1, space="PSUM"))
            dram = ctx.enter_context(tc.tile_pool(name="dram", bufs=1, space="DRAM"))

            # ---------------- DMA in (2 queues) ----------------
            lf_sb = sb.tile([BS, 2 * N + 1], F32)
            nc.sync.dma_start(out=lf_sb, in_=lf[:])
            mp_sb = sb.tile([BS, 14 * N], BF16)
            nc.sync.dma_start(out=mp_sb, in_=mp[:])
            tall = sb.tile([128, C_TALL], F32)
            nc.scalar.dma_start(out=tall, in_=tallt[:])
            short = sb.tile([16, C_SHORT], F32)
            nc.scalar.dma_start(out=short, in_=shortt[:])
            ego_sb = sb.tile([6, BS], F32)
            nc.scalar.dma_start(out=ego_sb, in_=egoT[0:6, :])
            egoM_sb = sb.tile([4, BS], F32)
            nc.scalar.dma_start(out=egoM_sb, in_=egoT[6:10, :])

            plane = lambda f: mp_sb[:, f * N:(f + 1) * N]
            loc = lf_sb[:, 0:N]
            flagc = lf_sb[:, N:2 * N]
            subj_id = lf_sb[:, 2 * N:2 * N + 1]
            subj_loc = lf_sb[:, 0:1]

            # ---------------- constants ----------------
            ones_col = sb.tile([128, 1], F32)
            nc.gpsimd.memset(ones_col, 1.0)
            ones_row = sb.tile([1, 128], F32)
            nc.gpsimd.memset(ones_row, 1.0)
            ident = sb.tile([128, 128], F32)
            make_identity(nc, ident)
            eps_col = sb.tile([128, 1], F32)
            nc.gpsimd.memset(eps_col, 1.0e-5)

            # ---------------- masks -> score accumulators (bf16) ----------------
            acc = {}
            geM = sb.tile([BS, N], BF16)
            nc.vector.tensor_scalar(geM, loc, subj_loc, NEG, op0=ALU.is_ge, op1=ALU.mult)
            nfMv = sb.tile([BS, N], BF16)
            nc.vector.tensor_scalar(nfMv, flagc, 1.0e9, NEG, op0=ALU.mult, op1=ALU.add)
            acc['u'] = sb.tile([BS, N], BF16, tag="accu", name="accu")
            nc.vector.tensor_tensor(acc['u'], geM, nfMv, op=ALU.min)
            acc['p'] = sb.tile([BS, N], BF16, tag="accp", name="accp")
            nc.vector.tensor_scalar(acc['p'], flagc, NEG, None, op0=ALU.mult)
            leM = sb.tile([BS, N], BF16)
            nc.vector.tensor_scalar(leM, loc, subj_loc, NEG, op0=ALU.is_le, op1=ALU.mult)
            acc['d'] = sb.tile([BS, N], BF16, tag="accd", name="accd")
            nc.vector.tensor_tensor(acc['d'], leM, nfMv, op=ALU.min)

            # ---------------- qk (PE) ----------------
            qk_ps = ps_sm.tile([BS, 35], F32, tag="sm", name="qk_ps")
            nc.tensor.matmul(qk_ps, ego_sb, short[0:6, WC:WC + 35],
                             start=True, stop=True)
            qk_sb = sb.tile([BS, 35], F32)
            nc.scalar.activation(qk_sb, qk_ps, ACTF.Copy, bias=0.0, scale=1.0)

            # ---------------- ego MLP early (PE otherwise idle) ----------------
            q1T, q2T = [], []
            for j, (w0, w1c) in enumerate(VC):
                pc = w1c - w0
                qp = ps_big.tile([pc, BS], F32, tag="big", name="qp")
                nc.tensor.matmul(qp, short[0:4, EW1 + w0:EW1 + w1c], egoM_sb,
                                 start=True, stop=True)
                qs = sb.tile([pc, BS], F32, tag=f"q1T{j}", name=f"q1T{j}")
                nc.scalar.activation(qs, qp, ACTF.Relu,
                                     bias=tall[0:pc, EB1 + j:EB1 + j + 1], scale=1.0)
                q1T.append(qs)
            for j, (w0, w1c) in enumerate(VC):
                pc = w1c - w0
                qp = ps_big.tile([pc, BS], F32, tag="big", name="qp2")
                for i, (c0, c1) in enumerate(VC):
                    nc.tensor.matmul(qp, tall[0:c1 - c0, EW2 + i * 200 + w0:EW2 + i * 200 + w1c],
                                     q1T[i], start=(i == 0), stop=(i == 1))
                qs = sb.tile([pc, BS], F32, tag=f"q2T{j}", name=f"q2T{j}")
                nc.scalar.activation(qs, qp, ACTF.Relu,
                                     bias=tall[0:pc, EB2 + j:EB2 + j + 1], scale=1.0)
                q2T.append(qs)
            G = psg.tile([BS, 1], F32)
            for i, (c0, c1) in enumerate(VC):
                nc.tensor.matmul(G, q2T[i], tall[0:c1 - c0, EW3 + i:EW3 + i + 1],
                                 start=(i == 0), stop=False, skip_group_check=True)
            nc.tensor.matmul(G, ones_row, short[0:1, CB:CB + 1], start=False,
                             stop=False, skip_group_check=True)

            # ---------------- scores (DVE, bf16) ----------------
            for s, nf, j0 in SEG:
                for f in range(nf):
                    nc.vector.scalar_tensor_tensor(
                        acc[s], plane(f), qk_sb[:, j0 + f:j0 + f + 1], acc[s],
                        op0=ALU.mult, op1=ALU.add)

            if STAGE <= 1:
                g_sb = sb.tile([BS, 1], F32, name="g_sb")
                nc.vector.tensor_copy(g_sb, acc['u'][:, 0:1])
                nc.sync.dma_start(out=out[:], in_=g_sb)
                return nc

            # ---------------- softmax exp (ACT) + recip (DVE) ----------------
            # d's DVE recip chain is deferred past pool_u/pool_p so the late
            # GpSimd d-scores don't stall the Vector queue.
            w_t, rs_t, wsum1 = {}, {}, {}
            se_t = {}
            for s in ['u', 'd', 'p']:  # scalar queue order = score finish order
                w_t[s] = sb.tile([BS, N], BF16, tag=f"w{s}", name=f"w{s}")
                se_t[s] = sb.tile([BS, 1], F32, tag=f"se{s}", name=f"se{s}")
                nc.scalar.activation(w_t[s], acc[s], ACTF.Exp, bias=0.0, scale=1.0,
                                     accum_out=se_t[s])

            def recip_chain(s):
                seb = sb.tile([BS, 1], F32, tag=f"seb{s}", name=f"seb{s}")
                nc.vector.tensor_scalar_add(seb, se_t[s], 1.0e-30)
                rs_t[s] = sb.tile([BS, 1], F32, tag=f"rs{s}", name=f"rs{s}")
                nc.vector.reciprocal(rs_t[s], seb)
                wsum1[s] = sb.tile([BS, 1], F32, tag=f"ws{s}", name=f"ws{s}")
                nc.vector.tensor_tensor(wsum1[s], se_t[s], rs_t[s], op=ALU.mult)

            for s in ['u', 'd', 'p']:
                recip_chain(s)

            if STAGE <= 2:
                g_sb = sb.tile([BS, 1], F32, name="g_sb")
                nc.vector.tensor_copy(g_sb, rs_t['u'])
                nc.sync.dma_start(out=out[:], in_=g_sb)
                return nc

            # ---------------- pool (DVE, bf16 STT + f32 accum) ----------------
            scr_v = sb.tile([BS, N], BF16)
            pool = {s: sb.tile([BS, 16], F32, tag=f"pool{s}", name=f"pool{s}")
                    for s, _, _ in SEG}
            for s, nf, _ in SEG:
                for f in range(nf):
                    nc.vector.scalar_tensor_tensor(
                        scr_v, plane(f), 1.0, w_t[s], op0=ALU.mult, op1=ALU.mult,
                        accum_out=pool[s][:, f:f + 1])
            # normalize + subject row + ones column
            for s, nf, _ in SEG:
                nc.vector.tensor_scalar_mul(pool[s][:, 0:nf], pool[s][:, 0:nf], rs_t[s])
                if nf < 14:
                    nc.vector.memset(pool[s][:, nf:14], 0.0)
                nc.vector.tensor_tensor(pool[s][:, 14:15], subj_id, wsum1[s], op=ALU.mult)
                nc.vector.memset(pool[s][:, 15:16], 1.0)

            if STAGE <= 3:
                g_sb = sb.tile([BS, 1], F32, name="g_sb")
                nc.vector.tensor_copy(g_sb, pool['u'][:, 0:1])
                nc.sync.dma_start(out=out[:], in_=g_sb)
                return nc

            # ---------------- Gram matrices -> collective ----------------
            gt_ps = ps_sm.tile([16, 48], F32, tag="sm", name="gt_ps")
            for si, (s, nf, _) in enumerate(SEG):
                nc.tensor.matmul(gt_ps[:, si * 16:(si + 1) * 16], pool[s], pool[s],
                                 start=True, stop=True)
            gt_sb = sb.tile([16, 48], F32)
            nc.vector.tensor_copy(gt_sb, gt_ps)
            in_b = dram.tile([16, 48], F32)
            nc.sync.dma_start(out=in_b[:], in_=gt_sb)
            if os.environ.get("NO_CC"):
                out_b = dram.tile([16, 48], F32, addr_space="Shared")
                nc.sync.dma_start(out=out_b[:], in_=in_b[:])
            elif AG:
                out_b = dram.tile([128, 48], F32, addr_space="Shared")
                nc.gpsimd.collective_compute(
                    "AllGather", ALU.bypass, ins=[in_b[:]], outs=[out_b[:]],
                    replica_groups=[list(range(NC))])
            else:
                out_b = dram.tile([16, 48], F32, addr_space="Shared")
                nc.gpsimd.collective_compute(
                    "AllReduce", ALU.add, ins=[in_b[:]], outs=[out_b[:]],
                    replica_groups=[list(range(NC))])

            # ---------------- overlap window: pool^T and x^T (indep of CC) ----------------
            poolT = {}
            for s, nf, _ in SEG:
                pT = ps_sm.tile([16, BS], F32, tag="sm", name="pT")
                nc.tensor.transpose(pT, pool[s], ident)
                poolT[s] = sb.tile([16, BS], F32, tag=f"pT{s}", name=f"pT{s}")
                nc.scalar.activation(poolT[s], pT, ACTF.Copy, bias=0.0, scale=1.0)
            xT = {}
            for si, (s, nf, _) in enumerate(SEG):
                xT[s] = []
                for i, (c0, c1) in enumerate(VC):
                    xps = ps_big.tile([c1 - c0, BS], F32, tag="big", name="xps")
                    nc.tensor.matmul(xps, short[0:15, WV + si * V + c0:WV + si * V + c1],
                                     poolT[s][0:15, :], start=True, stop=True)
                    xsb = sb.tile([c1 - c0, BS], F32, tag=f"xT{s}{i}", name=f"xT{s}{i}")
                    nc.scalar.activation(xsb, xps, ACTF.Copy, bias=0.0, scale=1.0)
                    xT[s].append(xsb)

            # ---------------- collective readback ----------------
            if AG and not os.environ.get("NO_CC"):
                gg8 = sb.tile([128, 48], F32)
                nc.sync.dma_start(out=gg8, in_=out_b[:])
                t01 = sb.tile([16, 48], F32)
                nc.vector.tensor_tensor(t01, gg8[0:16, :], gg8[16:32, :], op=ALU.add)
                t23 = sb.tile([16, 48], F32)
                nc.vector.tensor_tensor(t23, gg8[32:48, :], gg8[48:64, :], op=ALU.add)
                t45 = sb.tile([16, 48], F32)
                nc.vector.tensor_tensor(t45, gg8[64:80, :], gg8[80:96, :], op=ALU.add)
                t67 = sb.tile([16, 48], F32)
                nc.vector.tensor_tensor(t67, gg8[96:112, :], gg8[112:128, :], op=ALU.add)
                nc.vector.tensor_tensor(t01, t01, t23, op=ALU.add)
                nc.vector.tensor_tensor(t45, t45, t67, op=ALU.add)
                gg = sb.tile([16, 48], F32)
                nc.vector.tensor_tensor(gg, t01, t45, op=ALU.add)
            else:
                gg = sb.tile([16, 48], F32)
                nc.sync.dma_start(out=gg, in_=out_b[:])

            if STAGE <= 4:
                gb_ps = ps_sm.tile([BS, 1], F32, tag="sm", name="gb_ps")
                nc.tensor.matmul(gb_ps, ones_row, gg[0:1, 0:1], start=True, stop=True)
                g_sb = sb.tile([BS, 1], F32, name="g_sb")
                nc.vector.tensor_copy(g_sb, gb_ps)
                nc.sync.dma_start(out=out[:], in_=g_sb)
                return nc

            # ---------------- per-feature stats from Gram ----------------
            # T1_s[i,v] = sum_j G[j,i] wv[j,v]; row 15 = sum_b x[:,v]
            prod = []
            for si, (s, nf, _) in enumerate(SEG):
                t1 = ps_big.tile([16, V], F32, tag="big", name="t1")
                nc.tensor.matmul(t1, gg[0:15, si * 16:(si + 1) * 16],
                                 short[0:15, WV + si * V:WV + (si + 1) * V],
                                 start=True, stop=True)
                pr = sb.tile([15, V], F32, tag=f"prod{si}", name=f"prod{si}")
                nc.vector.tensor_tensor(pr, short[0:15, WV + si * V:WV + (si + 1) * V],
                                        t1[0:15, :], op=ALU.mult)
                prod.append(pr)

            s3_t, t03_t = [], []
            for j, (c0, c1) in enumerate(VC):
                pc = c1 - c0
                stp = ps_sm.tile([pc, 6], F32, tag="sm", name=f"stp{j}")
                for si in range(3):
                    nc.tensor.matmul(stp[:, si:si + 1],
                                     short[0:15, WV + si * V + c0:WV + si * V + c1],
                                     gg[0:15, si * 16 + 15:si * 16 + 16],
                                     start=True, stop=True)
                    nc.tensor.matmul(stp[:, 3 + si:4 + si], prod[si][:, c0:c1],
                                     ones_col[0:15, :], start=True, stop=True)
                st = sb.tile([pc, 6], F32, tag=f"st{j}", name=f"st{j}")
                nc.vector.tensor_scalar(st, stp, 1.0 / B, None, op0=ALU.mult)
                sq = sb.tile([pc, 3], F32, tag=f"sq{j}", name=f"sq{j}")
                nc.vector.tensor_tensor(sq, st[:, 0:3], st[:, 0:3], op=ALU.mult)
                var = sb.tile([pc, 3], F32, tag=f"var{j}", name=f"var{j}")
                nc.vector.tensor_tensor(var, st[:, 3:6], sq, op=ALU.subtract)
                # rstd = exp(-0.5*ln(var+eps)) -- stays in the exp/ln table set
                lnv = sb.tile([pc, 3], F32, tag=f"lnv{j}", name=f"lnv{j}")
                nc.scalar.activation(lnv, var, ACTF.Ln, bias=eps_col[0:pc, :], scale=1.0)
                rstd = sb.tile([pc, 3], F32, tag=f"rstd{j}", name=f"rstd{j}")
                nc.scalar.activation(rstd, lnv, ACTF.Exp, bias=0.0, scale=-0.5)
                gam_b = tall[0:pc, GAM + j:GAM + j + 1]
                gam_b = bass.AP(tensor=gam_b.tensor, offset=gam_b.offset,
                                ap=[gam_b.ap[0], [0, 3]])
                bet_b = tall[0:pc, BET + j:BET + j + 1]
                bet_b = bass.AP(tensor=bet_b.tensor, offset=bet_b.offset,
                                ap=[bet_b.ap[0], [0, 3]])
                s3 = sb.tile([pc, 3], F32, tag=f"s3{j}", name=f"s3{j}")
                nc.vector.tensor_tensor(s3, rstd, gam_b, op=ALU.mult)
                z3 = sb.tile([pc, 3], F32, tag=f"z3{j}", name=f"z3{j}")
                nc.vector.tensor_tensor(z3, st[:, 0:3], s3, op=ALU.mult)
                t03 = sb.tile([pc, 3], F32, tag=f"t03{j}", name=f"t03{j}")
                nc.vector.tensor_tensor(t03, bet_b, z3, op=ALU.subtract)
                s3_t.append(s3)
                t03_t.append(t03)

            if STAGE <= 5:
                g_sb = sb.tile([BS, 1], F32, name="g_sb")
                nc.vector.tensor_copy(g_sb, s3_t[0][:, 0:1])
                nc.sync.dma_start(out=out[:], in_=g_sb)
                return nc

            # ---------------- BN folded into head inputs ----------------
            xs = {}  # xs[k][i] = s3 * xT  (BN scale applied to activations)
            for k, s in enumerate(['u', 'd', 'p']):
                xs[k] = []
                for i, (c0, c1) in enumerate(VC):
                    t = sb.tile([c1 - c0, BS], F32, tag=f"xs{k}{i}", name=f"xs{k}{i}")
                    nc.vector.tensor_scalar_mul(t, xT[s][i], s3_t[i][:, k:k + 1])
                    xs[k].append(t)
            # b1' = b1 + t0 @ W1  (raw W1), [pc,3] per out-chunk
            B1 = []
            for j, (w0, w1c) in enumerate(VC):
                pc = w1c - w0
                bt = sb.tile([pc, 3], F32, tag=f"B1{j}", name=f"B1{j}")
                for k in range(3):
                    bp = ps_sm.tile([pc, 1], F32, tag="sm", name="bp")
                    for i, (c0, c1) in enumerate(VC):
                        nc.tensor.matmul(bp,
                                         tall[0:c1 - c0, W1 + i * 600 + k * V + w0:W1 + i * 600 + k * V + w1c],
                                         t03_t[i][:, k:k + 1],
                                         start=(i == 0), stop=(i == 1))
                    nc.vector.tensor_copy(bt[:, k:k + 1], bp)
                nc.vector.tensor_tensor(bt, bt, tall[0:pc, B1T + j * 3:B1T + j * 3 + 3],
                                        op=ALU.add)
                B1.append(bt)

            if STAGE <= 6:
                g_sb = sb.tile([BS, 1], F32, name="g_sb")
                nc.vector.tensor_copy(g_sb, B1[0][:, 0:1])
                nc.sync.dma_start(out=out[:], in_=g_sb)
                return nc

            # ---------------- heads: elu+1 folded, accumulate into G ----------------
            nmm = 0
            for k in range(3):
                for j, (w0, w1c) in enumerate(VC):
                    pc = w1c - w0
                    hp = ps_big.tile([pc, BS], F32, tag="big", name="hp")
                    for i, (c0, c1) in enumerate(VC):
                        nc.tensor.matmul(hp,
                                         tall[0:c1 - c0, W1 + i * 600 + k * V + w0:W1 + i * 600 + k * V + w1c],
                                         xs[k][i], start=(i == 0), stop=(i == 1))
                    eh = sb.tile([pc, BS], F32, tag=f"eh{j}", name=f"eh{j}")
                    nc.scalar.activation(eh, hp, ACTF.Exp, bias=B1[j][:, k:k + 1],
                                         scale=1.0)
                    rh = sb.tile([pc, BS], F32, tag=f"rh{j}", name=f"rh{j}")
                    nc.vector.tensor_scalar(rh, hp, B1[j][:, k:k + 1], 0.0,
                                            op0=ALU.add, op1=ALU.max)
                    ht = sb.tile([pc, BS], F32, tag=f"ht{j}", name=f"ht{j}")
                    nc.vector.scalar_tensor_tensor(ht, eh, 1.0, rh,
                                                   op0=ALU.min, op1=ALU.add)
                    nc.tensor.matmul(G, ht, tall[0:pc, W2 + j * 3 + k:W2 + j * 3 + k + 1],
                                     start=False, stop=(k == 2 and j == 1),
                                     skip_group_check=True)
                    nmm += 1

            g_sb = sb.tile([BS, 1], F32)
            nc.vector.tensor_copy(g_sb, G)
            nc.sync.dma_start(out=out[:], in_=g_sb)

    nc.finalize()
    return nc


def prep_inputs(inputs):
    """Host-side prep: layout/dtype for data, constant folding for weights."""
    m = np.ascontiguousarray(inputs["merged"], dtype=np.float32)
    a = np.ascontiguousarray(inputs["a"], dtype=np.float32)
    f32 = lambda x: np.ascontiguousarray(x, dtype=np.float32)

    up_Wq, up_Wk, up_Wv = inputs["up_Wq"], inputs["up_Wk"], inputs["up_Wv"]
    dn_Wq, dn_Wk, dn_Wv = inputs["dn_Wq"], inputs["dn_Wk"], inputs["dn_Wv"]
    pv_Wq, pv_Wk, pv_Wv = inputs["pv_Wq"], inputs["pv_Wk"], inputs["pv_Wv"]
    t_W1, t_b1, t_W2, t_b2 = inputs["t_W1"], inputs["t_b1"], inputs["t_W2"], inputs["t_b2"]
    e_W1, e_b1, e_W2, e_b2 = inputs["e_W1"], inputs["e_b1"], inputs["e_W2"], inputs["e_b2"]
    e_W3, e_b3 = inputs["e_W3"], inputs["e_b3"]
    gamma, beta = inputs["gamma"], inputs["beta"]

    # wc = (Wq @ Wk^T)[1:7] * scale per segment (weight-only constant)
    wc = np.zeros((6, 35), np.float32)
    for (Wq, Wk), o0, nf in [((up_Wq, up_Wk), 0, 14), ((dn_Wq, dn_Wk), 14, 14),
                             ((pv_Wq, pv_Wk), 28, 7)]:
        wc[:, o0:o0 + nf] = (np.asarray(Wq, np.float32) @ np.asarray(Wk, np.float32).T)[1:7] * SCALE
    # wv with pv zero-padded to 14 rows, + ext row 14 = -(row0+row7)
    wv = np.zeros((16, 3 * V), np.float32)
    wv[0:14, 0:V] = up_Wv
    wv[0:14, V:2 * V] = dn_Wv
    wv[0:7, 2 * V:3 * V] = pv_Wv
    wv[14] = -(wv[0] + wv[7])

    def chunk2(arr):  # [200, c] -> [128, 2c]
        arr = np.asarray(arr, np.float32)
        c = arr.shape[1]
        o = np.zeros((128, 2 * c), np.float32)
        o[:, :c] = arr[0:128]
        o[0:72, c:] = arr[128:200]
        return o

    w1 = np.concatenate([t_W1[0], t_W1[1], t_W1[2]], axis=1)          # [200,600]
    tall = np.concatenate([
        chunk2(w1), chunk2(e_W2), chunk2(np.asarray(t_W2)[:, :, 0].T),
        chunk2(np.asarray(t_b1).T), chunk2(e_W3),
        chunk2(np.asarray(e_b1)[:, None]), chunk2(np.asarray(e_b2)[:, None]),
        chunk2(np.asarray(gamma)[:, None]), chunk2(np.asarray(beta)[:, None]),
    ], axis=1)
    assert tall.shape == (128, C_TALL), tall.shape

    short = np.zeros((16, C_SHORT), np.float32)
    short[:, WV:WV + 3 * V] = wv
    short[0:6, WC:WC + 35] = wc
    short[0:4, EW1:EW1 + V] = e_W1
    short[0, CB] = float(np.sum(t_b2)) + float(np.sum(e_b3)) - float(np.sum(t_W2))

    # data: loc/flag/subj_id f32, feature planes bf16
    lf_full = np.concatenate([m[:, :, 2], m[:, :, 14], m[:, 0, 0:1]], axis=1)
    feats = np.ascontiguousarray(
        m[:, :, 0:14].transpose(0, 2, 1).reshape(B, 14 * N)).astype(ml_dtypes.bfloat16)

    in_maps = []
    for c in range(NC):
        sl = slice(c * BS, (c + 1) * BS)
        sh = m[sl]
        egoT = np.zeros((10, BS), np.float32)
        egoT[0:5] = sh[:, 0, 1:6].T
        egoT[5] = a[sl]
        egoT[6:9] = sh[:, 0, 3:6].T
        egoT[9] = a[sl]
        in_maps.append(dict(
            lf=f32(lf_full[sl]), mp=np.ascontiguousarray(feats[sl]),
            tall=tall, short=short, egoT=f32(egoT)))
    return in_maps


def _build():
    nc = build_nc()
    if not nc.is_finalized():
        nc.finalize()
    return nc


def kernel(**inputs):
    if "nc" not in _cache:
        _cache["nc"] = _build()
    nc = _cache["nc"]
    in_maps = prep_inputs(inputs)
    r = run_bass_kernel_spmd(nc, in_maps, list(range(NC)), trace=False)
    _cache["last"] = r
    out = np.concatenate([r.results[c]["out"] for c in range(NC)], axis=0)
    return out.reshape(-1, 1).astype(np.float32)
